# revision 2
# baseline (speedup 1.0000x reference)
"""Trainium2 Bass kernel for a 3-layer relu-LSTM classifier.

Architecture (per core, data-parallel over batch across 8 cores, B=16 each):
  x = emb[tokens]                       (indirect-DMA gather, 128 tokens/block)
  xg_l = x_l @ W_l + b_l  (bulk, PE)    -> DRAM, "folded transposed" layout
  recurrence per step (For_i):  g.T = U_l.T @ h.T  (PE, bf16 weights)
      gates/state kept as [128 partitions, nk*16] folded layout so the
      DVE/ACT elementwise ops use all 128 partitions.
  dense head on-device, output sigmoid [16] f32 per core.

Self-contained: hardcodes all shapes; host side only reformats weights
(permutation/fold/bf16 cast) and shards tokens.
"""

import os

import numpy as np
import ml_dtypes

BF16 = ml_dtypes.bfloat16

# Model dims
NCORES = 8
B_TOT, T = 128, 512
B = B_TOT // NCORES  # 16
VOCAB, EMB_D = 5000, 300
EMB_PAD = 384  # padded to 3*128
UNITS = [256, 512, 256]
DENSE = 64

# Per-layer derived dims
# layer l: u, d_in(padded), nk = u//128 (contraction tiles / h fold slabs),
# nm = 4*nk (gate m-tiles), F = nk*16 (fold width), FW = 4*F (g fold width)
LCFG = []
_d = EMB_PAD
for _u in UNITS:
    _nk = _u // 128
    LCFG.append(dict(u=_u, d=_d, nkw=_d // 128, nk=_nk, nm=4 * _nk,
                     F=_nk * 16, FW=4 * _nk * 16))
    _d = _u

TC = 32          # time-steps per bulk-projection chunk (N = TC*16 = 512)
NCHUNK = T // TC
UNROLL = 4       # rec loop half-steps per For_i body (must be even)
STAGGERED = os.environ.get("K_STAGGERED", "1") == "1"

_CACHE = {}
LAST_RESULT = None  # BassKernelResults of the most recent run (for test.py)


def gate_perm(u):
    """Column permutation of [i f cc o]-ordered 4u gate dim into our
    m-tile order: blocks (f, cc, i, o), each block j-minor over u//128."""
    nk = u // 128
    base = [1, 2, 0, 3]  # block idx -> keras gate idx (i=0, f=1, cc=2, o=3)
    perm = np.empty(4 * u, dtype=np.int64)
    for blk in range(4):
        for j in range(nk):
            m = blk * nk + j
            perm[m * 128:(m + 1) * 128] = base[blk] * u + j * 128 + np.arange(128)
    return perm


def fold_lhs(Wp, nkt, nm):
    """[nkt*128, nm*128] -> [128, nkt*nm*128] with tile (k, m) at cols
    ((k*nm)+m)*128."""
    K, M = Wp.shape
    assert K == nkt * 128 and M == nm * 128, (Wp.shape, nkt, nm)
    return np.ascontiguousarray(
        Wp.reshape(nkt, 128, nm, 128).transpose(1, 0, 2, 3).reshape(128, nkt * nm * 128)
    )


def prep_weights(inputs):
    """Host-side reformatting of the model weights (shared by all cores)."""
    f32 = lambda x: np.asarray(x, dtype=np.float32)
    out = {}
    perms = [gate_perm(u) for u in UNITS]
    W0 = np.zeros((EMB_PAD, 4 * UNITS[0]), np.float32)
    W0[:EMB_D] = f32(inputs["W0"])
    Ws = [W0, f32(inputs["W1"]), f32(inputs["W2"])]
    for l in range(3):
        cfg = LCFG[l]
        p = perms[l]
        out[f"w{l}"] = fold_lhs(Ws[l][:, p], cfg["nkw"], cfg["nm"]).astype(BF16)
        out[f"u{l}"] = fold_lhs(f32(inputs[f"U{l}"])[:, p], cfg["nk"], cfg["nm"]).astype(BF16)
        out[f"b{l}"] = np.ascontiguousarray(
            f32(inputs[f"b{l}"])[p].reshape(cfg["nm"], 128).T)
    Wd = f32(inputs["Wd"])  # [256, 64]
    out["wd"] = np.concatenate([Wd[0:128], Wd[128:256]], axis=1).astype(BF16)  # [128,128]
    out["bd"] = f32(inputs["bd"])           # [64]
    out["wc"] = f32(inputs["Wc"]).astype(BF16)  # [64, 1]
    out["bc"] = f32(inputs["bc"])           # [1]
    return out


def build_program():
    from concourse import bacc
    import concourse.mybir as mybir
    import concourse.tile as tile
    from concourse.bass import ds

    FP32 = mybir.dt.float32
    BF = mybir.dt.bfloat16
    AF = mybir.ActivationFunctionType
    ALU = mybir.AluOpType

    nc = bacc.Bacc(None, target_bir_lowering=False)

    # ---- DRAM parameters ------------------------------------------------
    tok_d = nc.declare_dram_parameter("tokens_tb", [T * B], mybir.dt.int32, isOutput=False)
    emb_d = nc.declare_dram_parameter("emb", [VOCAB, EMB_D], FP32, isOutput=False)
    wp = {}
    for l in range(3):
        cfg = LCFG[l]
        wp[f"w{l}"] = nc.declare_dram_parameter(f"w{l}", [128, cfg["nkw"] * cfg["nm"] * 128], BF, isOutput=False)
        wp[f"u{l}"] = nc.declare_dram_parameter(f"u{l}", [128, cfg["nk"] * cfg["nm"] * 128], BF, isOutput=False)
        wp[f"b{l}"] = nc.declare_dram_parameter(f"b{l}", [128, cfg["nm"]], FP32, isOutput=False)
    wd_d = nc.declare_dram_parameter("wd", [128, 128], BF, isOutput=False)
    bd_d = nc.declare_dram_parameter("bd", [DENSE], FP32, isOutput=False)
    wc_d = nc.declare_dram_parameter("wc", [DENSE, 1], BF, isOutput=False)
    bc_d = nc.declare_dram_parameter("bc", [1], FP32, isOutput=False)
    out_d = nc.declare_dram_parameter("out", [B], FP32, isOutput=True)

    # ---- internal DRAM scratch -----------------------------------------
    xg_d = [nc.dram_tensor(f"xg{l}", [128, T * LCFG[l]["FW"]], FP32) for l in range(3)]

    from concourse.masks import make_identity

    with tile.TileContext(nc) as tc:
        stk = []

        def pool(name, bufs, space="SBUF"):
            return tc.tile_pool(name=name, bufs=bufs, space=space)

        with pool("const", 1) as constp:
            ident = constp.tile([128, 128], FP32)
            make_identity(nc, ident[:])
            tok_sb = constp.tile([128, (T * B) // 128], mybir.dt.int32)
            nc.sync.dma_start(tok_sb[:], tok_d[:].rearrange("(i p) -> p i", p=128))
            bias_sb = []
            for l in range(3):
                bt = constp.tile([128, LCFG[l]["nm"]], FP32, tag=f"bias{l}")
                nc.sync.dma_start(bt[:], wp[f"b{l}"][:])
                bias_sb.append(bt)
            wd_sb = constp.tile([128, 128], BF)
            nc.sync.dma_start(wd_sb[:], wd_d[:])
            bd_sb = constp.tile([DENSE, 1], FP32)
            nc.sync.dma_start(bd_sb[:], bd_d[:])
            wc_sb = constp.tile([DENSE, 1], BF)
            nc.sync.dma_start(wc_sb[:], wc_d[:])
            bc_sb = constp.tile([1, 1], FP32)
            nc.sync.dma_start(bc_sb[:], bc_d[:])

            # ============ Phase A: gather + transpose -> xT =============
            NTOK = T * B           # 8192
            NBLK = NTOK // 128     # 64
            with pool("xT", 1) as xtp:
                xT = xtp.tile([128, 3 * NTOK], BF)
                # zero slab k=2 (rows 44: stay zero; 0:44 overwritten below)
                nc.gpsimd.memset(xT[:, 2 * NTOK:3 * NTOK], 0.0)
                with nc.named_scope("gather_transpose"):
                    with pool("gath", 3) as gp, pool("tps", 2, "PSUM") as tpp:
                        for blk in range(NBLK):
                            xb = gp.tile([128, EMB_PAD], FP32, tag="xb")
                            import concourse.bass as bass_mod
                            nc.gpsimd.indirect_dma_start(
                                out=xb[:, 0:EMB_D], out_offset=None,
                                in_=emb_d[:, :],
                                in_offset=bass_mod.IndirectOffsetOnAxis(
                                    ap=tok_sb[:, blk:blk + 1], axis=0),
                            )
                            for k in range(3):
                                tps = tpp.tile([128, 128], FP32, tag="tps")
                                nc.tensor.transpose(tps[:], xb[:, 128 * k:128 * (k + 1)], ident[:])
                                rows = 128 if k < 2 else 44
                                nc.vector.tensor_copy(
                                    out=xT[0:rows, k * NTOK + 128 * blk: k * NTOK + 128 * (blk + 1)],
                                    in_=tps[0:rows, :])

                # ============ Phase B: xg0 bulk =============
                _bulk_proj(nc, tc, pool, 0, wp["w0"], bias_sb[0], xg_d[0],
                           rhs_fn=lambda k, c: xT[:, k * NTOK + c * 512: k * NTOK + (c + 1) * 512])

            # ============ Phase C: L0 recurrence =============
            with pool("seq0", 1) as sq0:
                h0_seq = sq0.tile([128, (T + 1) * LCFG[0]["F"]], BF)
                _recurrence(nc, tc, pool, 0, wp["u0"], xg_d[0], h0_seq, ds)

                # ============ Phase D: xg1 bulk =============
                F0 = LCFG[0]["F"]
                h0r = h0_seq[:].rearrange("p (s w) -> p s w", w=F0)
                _bulk_proj(nc, tc, pool, 1, wp["w1"], bias_sb[1], xg_d[1],
                           rhs_fn=lambda k, c: h0r[:, c * TC + 1: (c + 1) * TC + 1, k * 16:(k + 1) * 16])

            # ============ Phase E: L1 recurrence =============
            with pool("seq1", 1) as sq1:
                h1_seq = sq1.tile([128, (T + 1) * LCFG[1]["F"]], BF)
                _recurrence(nc, tc, pool, 1, wp["u1"], xg_d[1], h1_seq, ds)

                # ============ Phase F: xg2 bulk =============
                F1 = LCFG[1]["F"]
                h1r = h1_seq[:].rearrange("p (s w) -> p s w", w=F1)
                _bulk_proj(nc, tc, pool, 2, wp["w2"], bias_sb[2], xg_d[2],
                           rhs_fn=lambda k, c: h1r[:, c * TC + 1: (c + 1) * TC + 1, k * 16:(k + 1) * 16])

            # ============ Phase G: L2 recurrence =============
            hb2 = _recurrence(nc, tc, pool, 2, wp["u2"], xg_d[2], None, ds)

            # ============ Phase H: dense head =============
            with nc.named_scope("dense"):
                F2 = LCFG[2]["F"]
                with pool("dps", 1, "PSUM") as dpp:
                    psd = dpp.tile([DENSE, 16], FP32, tag="psd")
                    for k in range(2):
                        nc.tensor.matmul(psd[:], lhsT=wd_sb[:, 64 * k:64 * (k + 1)],
                                         rhs=hb2[:, F2 + 16 * k:F2 + 16 * (k + 1)],
                                         start=(k == 0), stop=(k == 1))
                    hd = constp.tile([DENSE, 16], BF, tag="hd")
                    nc.scalar.activation(hd[:], psd[:], AF.Relu, bias=bd_sb[:, 0:1])
                    psc = dpp.tile([1, 16], FP32, tag="psc")
                    nc.tensor.matmul(psc[:], lhsT=wc_sb[:], rhs=hd[:], start=True, stop=True)
                    outv = constp.tile([1, 16], FP32, tag="outv")
                    nc.scalar.activation(outv[:], psc[:], AF.Sigmoid, bias=bc_sb[0:1, 0:1])
                    nc.sync.dma_start(out_d[:], outv[0:1, :])

    nc.finalize()
    return nc


def _bulk_proj(nc, tc, pool, l, w_dram, bias_sb, xg_dram, rhs_fn):
    """xg_l[:, t*FW + m*16 + b] over a chunked [TC*16]-token loop.
    rhs_fn(k, chunk) -> [128, 512]-sized AP of the (transposed) layer input."""
    import concourse.mybir as mybir
    FP32 = mybir.dt.float32
    BF = mybir.dt.bfloat16
    ALU = mybir.AluOpType
    cfg = LCFG[l]
    nkw, nm, FW = cfg["nkw"], cfg["nm"], cfg["FW"]
    with nc.named_scope(f"xg{l}_bulk"):
        with pool(f"w{l}p", 1) as wpool, pool(f"xps{l}", 2, "PSUM") as xpp, \
                pool(f"stage{l}", 2) as stp:
            w_sb = wpool.tile([128, nkw * nm * 128], BF)
            nc.sync.dma_start(w_sb[:], w_dram[:])
            for c in range(NCHUNK):
                stage = stp.tile([128, TC * FW], FP32, tag="stage")
                stager = stage[:].rearrange("p (t w) -> p t w", w=FW)
                for m in range(nm):
                    ps = xpp.tile([128, 512], FP32, tag="xps")
                    for k in range(nkw):
                        nc.tensor.matmul(
                            ps[:], lhsT=w_sb[:, ((k * nm) + m) * 128:((k * nm) + m + 1) * 128],
                            rhs=rhs_fn(k, c), start=(k == 0), stop=(k == nkw - 1))
                    nc.vector.tensor_scalar(
                        out=stager[:, :, m * 16:(m + 1) * 16],
                        in0=ps[:].rearrange("p (t b) -> p t b", b=16),
                        scalar1=bias_sb[:, m:m + 1], scalar2=None, op0=ALU.add)
                nc.sync.dma_start(xg_dram[:, c * TC * FW:(c + 1) * TC * FW], stage[:])


def _recurrence(nc, tc, pool, l, u_dram, xg_dram, h_seq, ds):
    """Run the T-step LSTM recurrence for layer l. Returns the ping/pong h
    tile (final h in slab p=1). Writes h into h_seq slots 1..T if given."""
    import concourse.mybir as mybir
    FP32 = mybir.dt.float32
    BF = mybir.dt.bfloat16
    AF = mybir.ActivationFunctionType
    ALU = mybir.AluOpType
    cfg = LCFG[l]
    nk, nm, F, FW = cfg["nk"], cfg["nm"], cfg["F"], cfg["FW"]
    split_o = (l == 1)

    with nc.named_scope(f"rec{l}"):
        with pool(f"u{l}p", 1) as upool, pool(f"st{l}", 1) as statep:
            u_sb = upool.tile([128, nk * nm * 128], BF)
            nc.sync.dma_start(u_sb[:], u_dram[:])
            hb = statep.tile([128, 2 * F], BF, tag="hb")
            cbuf = statep.tile([128, F], FP32, tag="cb")
            nc.gpsimd.memset(hb[:], 0.0)
            nc.gpsimd.memset(cbuf[:], 0.0)
            if h_seq is not None:
                nc.gpsimd.memset(h_seq[:, 0:F], 0.0)

            tc.strict_bb_all_engine_barrier()

            with pool(f"rp{l}a", 2, "PSUM") as ppa, pool(f"rp{l}o", 2, "PSUM") as ppo, \
                    pool(f"rx{l}", 2) as xgp, pool(f"rg{l}", 2) as gsp, \
                    pool(f"rt{l}", 2) as tmp:
                hint = (mybir.EngineType.PE,) if l == 1 else ()
                with tc.For_i(0, T, UNROLL, staggered_reset=STAGGERED,
                              hint_engines=hint) as i:
                    for po in range(UNROLL):
                        p = po % 2
                        t_expr = i + po
                        xg = xgp.tile([128, FW], FP32, tag="xg")
                        nc.sync.dma_start(xg[:], xg_dram[:, ds(t_expr * FW, FW)])
                        if split_o:
                            psa = ppa.tile([128, 3 * F], FP32, tag="psa")
                            pso = ppo.tile([128, F], FP32, tag="pso")
                        else:
                            psa = ppa.tile([128, FW], FP32, tag="psa")
                            pso = None
                        for m in range(nm):
                            if split_o and m >= 3 * nk:
                                dst = pso[:, (m - 3 * nk) * 16:(m - 3 * nk + 1) * 16]
                            else:
                                dst = psa[:, m * 16:(m + 1) * 16]
                            for k in range(nk):
                                nc.tensor.matmul(
                                    dst, lhsT=u_sb[:, ((k * nm) + m) * 128:((k * nm) + m + 1) * 128],
                                    rhs=hb[:, (1 - p) * F + k * 16:(1 - p) * F + (k + 1) * 16],
                                    start=(k == 0), stop=(k == nk - 1))
                        gs = gsp.tile([128, FW], FP32, tag="gs")
                        if split_o:
                            nc.vector.tensor_add(out=gs[:, 0:3 * F], in0=psa[:], in1=xg[:, 0:3 * F])
                        else:
                            nc.vector.tensor_add(out=gs[:], in0=psa[:], in1=xg[:])
                        sf = tmp.tile([128, F], FP32, tag="sf")
                        nc.scalar.activation(sf[:], gs[:, 0:F], AF.Sigmoid)
                        si = tmp.tile([128, F], FP32, tag="si")
                        nc.scalar.activation(si[:], gs[:, 2 * F:3 * F], AF.Sigmoid)
                        c2 = tmp.tile([128, F], FP32, tag="c2")
                        nc.vector.tensor_mul(out=c2[:], in0=cbuf[:], in1=sf[:])
                        t1 = tmp.tile([128, F], FP32, tag="t1")
                        nc.vector.scalar_tensor_tensor(
                            out=t1[:], in0=gs[:, F:2 * F], scalar=0.0, in1=si[:],
                            op0=ALU.max, op1=ALU.mult)
                        nc.vector.tensor_add(out=cbuf[:], in0=c2[:], in1=t1[:])
                        if split_o:
                            nc.vector.tensor_add(out=gs[:, 3 * F:FW], in0=pso[:], in1=xg[:, 3 * F:FW])
                        so = tmp.tile([128, F], FP32, tag="so")
                        nc.scalar.activation(so[:], gs[:, 3 * F:FW], AF.Sigmoid)
                        nc.vector.scalar_tensor_tensor(
                            out=hb[:, p * F:(p + 1) * F], in0=cbuf[:], scalar=0.0,
                            in1=so[:], op0=ALU.max, op1=ALU.mult)
                        if h_seq is not None:
                            nc.sync.dma_start(h_seq[:, ds((t_expr + 1) * F, F)],
                                              hb[:, p * F:(p + 1) * F])
            return hb


def _get_program():
    if "nc" not in _CACHE:
        _CACHE["nc"] = build_program()
    return _CACHE["nc"]


def kernel(**inputs):
    global LAST_RESULT
    from concourse.bass_utils import run_bass_kernel_spmd

    nc = _get_program()
    w = prep_weights(inputs)
    tokens = np.asarray(inputs["tokens"], dtype=np.int32)  # [128, 512]

    in_maps = []
    for core in range(NCORES):
        tk = tokens[core * B:(core + 1) * B]          # [16, 512]
        tok_tb = np.ascontiguousarray(tk.T).reshape(-1)  # t-major: idx = t*16+b
        m = {"tokens_tb": tok_tb,
             "emb": np.asarray(inputs["emb"], dtype=np.float32)}
        m.update(w)
        in_maps.append(m)

    trace = os.environ.get("K_TRACE", "0") == "1"
    res = run_bass_kernel_spmd(nc, in_maps, list(range(NCORES)), trace=trace)
    LAST_RESULT = res
    out = np.concatenate([res.results[c]["out"].reshape(B, 1) for c in range(NCORES)], axis=0)
    return out.astype(np.float32)


# revision 4
# speedup vs baseline: 1.0033x; 1.0033x over previous
"""Trainium2 Bass kernel for a 3-layer relu-LSTM classifier.

Architecture (per core, data-parallel over batch across 8 cores, B=16 each):
  x = emb[tokens]                       (indirect-DMA gather, 128 tokens/block)
  xg_l = x_l @ W_l + b_l  (bulk, PE)    -> DRAM, "folded transposed" layout
  recurrence per step (For_i):  g.T = U_l.T @ h.T  (PE, bf16 weights)
      gates/state kept as [128 partitions, nk*16] folded layout so the
      DVE/ACT elementwise ops use all 128 partitions.
  dense head on-device, output sigmoid [16] f32 per core.

Self-contained: hardcodes all shapes; host side only reformats weights
(permutation/fold/bf16 cast) and shards tokens.
"""

import os

import numpy as np
import ml_dtypes

BF16 = ml_dtypes.bfloat16

# Model dims
NCORES = 8
B_TOT, T = 128, 512
B = B_TOT // NCORES  # 16
VOCAB, EMB_D = 5000, 300
EMB_PAD = 384  # padded to 3*128
UNITS = [256, 512, 256]
DENSE = 64

# Per-layer derived dims
# layer l: u, d_in(padded), nk = u//128 (contraction tiles / h fold slabs),
# nm = 4*nk (gate m-tiles), F = nk*16 (fold width), FW = 4*F (g fold width)
LCFG = []
_d = EMB_PAD
for _u in UNITS:
    _nk = _u // 128
    LCFG.append(dict(u=_u, d=_d, nkw=_d // 128, nk=_nk, nm=4 * _nk,
                     F=_nk * 16, FW=4 * _nk * 16))
    _d = _u

TC = 32          # time-steps per bulk-projection chunk (N = TC*16 = 512)
NCHUNK = T // TC
UNROLL = 4       # rec loop half-steps per For_i body (must be even)
STAGGERED = os.environ.get("K_STAGGERED", "1") == "1"

_CACHE = {}
LAST_RESULT = None  # BassKernelResults of the most recent run (for test.py)


def gate_perm(u):
    """Column permutation of [i f cc o]-ordered 4u gate dim into our
    m-tile order: blocks (f, cc, i, o), each block j-minor over u//128."""
    nk = u // 128
    base = [1, 0, 3, 2]  # block order (f, i, o, cc); keras gate idx (i=0, f=1, cc=2, o=3)
    perm = np.empty(4 * u, dtype=np.int64)
    for blk in range(4):
        for j in range(nk):
            m = blk * nk + j
            perm[m * 128:(m + 1) * 128] = base[blk] * u + j * 128 + np.arange(128)
    return perm


def fold_lhs(Wp, nkt, nm):
    """[nkt*128, nm*128] -> [128, nkt*nm*128] with tile (k, m) at cols
    ((k*nm)+m)*128."""
    K, M = Wp.shape
    assert K == nkt * 128 and M == nm * 128, (Wp.shape, nkt, nm)
    return np.ascontiguousarray(
        Wp.reshape(nkt, 128, nm, 128).transpose(1, 0, 2, 3).reshape(128, nkt * nm * 128)
    )


def prep_weights(inputs):
    """Host-side reformatting of the model weights (shared by all cores)."""
    f32 = lambda x: np.asarray(x, dtype=np.float32)
    out = {}
    perms = [gate_perm(u) for u in UNITS]
    W0 = np.zeros((EMB_PAD, 4 * UNITS[0]), np.float32)
    W0[:EMB_D] = f32(inputs["W0"])
    Ws = [W0, f32(inputs["W1"]), f32(inputs["W2"])]
    for l in range(3):
        cfg = LCFG[l]
        p = perms[l]
        out[f"w{l}"] = fold_lhs(Ws[l][:, p], cfg["nkw"], cfg["nm"]).astype(BF16)
        out[f"u{l}"] = fold_lhs(f32(inputs[f"U{l}"])[:, p], cfg["nk"], cfg["nm"]).astype(BF16)
        out[f"b{l}"] = np.ascontiguousarray(
            f32(inputs[f"b{l}"])[p].reshape(cfg["nm"], 128).T)
    Wd = f32(inputs["Wd"])  # [256, 64]
    out["wd"] = np.concatenate([Wd[0:128], Wd[128:256]], axis=1).astype(BF16)  # [128,128]
    out["bd"] = f32(inputs["bd"])           # [64]
    out["wc"] = f32(inputs["Wc"]).astype(BF16)  # [64, 1]
    out["bc"] = f32(inputs["bc"])           # [1]
    return out


def build_program():
    from concourse import bacc
    import concourse.mybir as mybir
    import concourse.tile as tile
    from concourse.bass import ds

    FP32 = mybir.dt.float32
    BF = mybir.dt.bfloat16
    AF = mybir.ActivationFunctionType
    ALU = mybir.AluOpType

    nc = bacc.Bacc(None, target_bir_lowering=False)

    # ---- DRAM parameters ------------------------------------------------
    tok_d = nc.declare_dram_parameter("tokens_tb", [T * B], mybir.dt.int32, isOutput=False)
    emb_d = nc.declare_dram_parameter("emb", [VOCAB, EMB_D], FP32, isOutput=False)
    wp = {}
    for l in range(3):
        cfg = LCFG[l]
        wp[f"w{l}"] = nc.declare_dram_parameter(f"w{l}", [128, cfg["nkw"] * cfg["nm"] * 128], BF, isOutput=False)
        wp[f"u{l}"] = nc.declare_dram_parameter(f"u{l}", [128, cfg["nk"] * cfg["nm"] * 128], BF, isOutput=False)
        wp[f"b{l}"] = nc.declare_dram_parameter(f"b{l}", [128, cfg["nm"]], FP32, isOutput=False)
    wd_d = nc.declare_dram_parameter("wd", [128, 128], BF, isOutput=False)
    bd_d = nc.declare_dram_parameter("bd", [DENSE], FP32, isOutput=False)
    wc_d = nc.declare_dram_parameter("wc", [DENSE, 1], BF, isOutput=False)
    bc_d = nc.declare_dram_parameter("bc", [1], FP32, isOutput=False)
    out_d = nc.declare_dram_parameter("out", [B], FP32, isOutput=True)

    # ---- internal DRAM scratch -----------------------------------------
    xg_d = [nc.dram_tensor(f"xg{l}", [128, T * LCFG[l]["FW"]], FP32) for l in range(3)]

    from concourse.masks import make_identity

    with tile.TileContext(nc) as tc:
        stk = []

        def pool(name, bufs, space="SBUF"):
            return tc.tile_pool(name=name, bufs=bufs, space=space)

        with pool("const", 1) as constp:
            ident = constp.tile([128, 128], FP32)
            make_identity(nc, ident[:])
            tok_sb = constp.tile([128, (T * B) // 128], mybir.dt.int32)
            nc.sync.dma_start(tok_sb[:], tok_d[:].rearrange("(i p) -> p i", p=128))
            bias_sb = []
            for l in range(3):
                bt = constp.tile([128, LCFG[l]["nm"]], FP32, tag=f"bias{l}")
                nc.sync.dma_start(bt[:], wp[f"b{l}"][:])
                bias_sb.append(bt)
            wd_sb = constp.tile([128, 128], BF)
            nc.sync.dma_start(wd_sb[:], wd_d[:])
            bd_sb = constp.tile([DENSE, 1], FP32)
            nc.sync.dma_start(bd_sb[:], bd_d[:])
            wc_sb = constp.tile([DENSE, 1], BF)
            nc.sync.dma_start(wc_sb[:], wc_d[:])
            bc_sb = constp.tile([1, 1], FP32)
            nc.sync.dma_start(bc_sb[:], bc_d[:])

            # ============ Phase A: gather + transpose -> xT =============
            NTOK = T * B           # 8192
            NBLK = NTOK // 128     # 64
            with pool("xT", 1) as xtp:
                xT = xtp.tile([128, 3 * NTOK], BF)
                # zero slab k=2 (rows 44: stay zero; 0:44 overwritten below)
                nc.gpsimd.memset(xT[:, 2 * NTOK:3 * NTOK], 0.0)
                with nc.named_scope("gather_transpose"):
                    with pool("gath", 3) as gp, pool("tps", 2, "PSUM") as tpp:
                        for blk in range(NBLK):
                            xb = gp.tile([128, EMB_PAD], FP32, tag="xb")
                            import concourse.bass as bass_mod
                            nc.gpsimd.indirect_dma_start(
                                out=xb[:, 0:EMB_D], out_offset=None,
                                in_=emb_d[:, :],
                                in_offset=bass_mod.IndirectOffsetOnAxis(
                                    ap=tok_sb[:, blk:blk + 1], axis=0),
                            )
                            for k in range(3):
                                tps = tpp.tile([128, 128], FP32, tag="tps")
                                nc.tensor.transpose(tps[:], xb[:, 128 * k:128 * (k + 1)], ident[:])
                                rows = 128 if k < 2 else 44
                                nc.vector.tensor_copy(
                                    out=xT[0:rows, k * NTOK + 128 * blk: k * NTOK + 128 * (blk + 1)],
                                    in_=tps[0:rows, :])

                # ============ Phase B: xg0 bulk =============
                _bulk_proj(nc, tc, pool, 0, wp["w0"], bias_sb[0], xg_d[0],
                           rhs_fn=lambda k, c: xT[:, k * NTOK + c * 512: k * NTOK + (c + 1) * 512])

            # ============ Phase C: L0 recurrence =============
            with pool("seq0", 1) as sq0:
                h0_seq = sq0.tile([128, (T + 1) * LCFG[0]["F"]], BF)
                _recurrence(nc, tc, pool, 0, wp["u0"], xg_d[0], h0_seq, ds)

                # ============ Phase D: xg1 bulk =============
                F0 = LCFG[0]["F"]
                h0r = h0_seq[:].rearrange("p (s w) -> p s w", w=F0)
                _bulk_proj(nc, tc, pool, 1, wp["w1"], bias_sb[1], xg_d[1],
                           rhs_fn=lambda k, c: h0r[:, c * TC + 1: (c + 1) * TC + 1, k * 16:(k + 1) * 16])

            # ============ Phase E: L1 recurrence =============
            with pool("seq1", 1) as sq1:
                h1_seq = sq1.tile([128, (T + 1) * LCFG[1]["F"]], BF)
                _recurrence(nc, tc, pool, 1, wp["u1"], xg_d[1], h1_seq, ds)

                # ============ Phase F: xg2 bulk =============
                F1 = LCFG[1]["F"]
                h1r = h1_seq[:].rearrange("p (s w) -> p s w", w=F1)
                _bulk_proj(nc, tc, pool, 2, wp["w2"], bias_sb[2], xg_d[2],
                           rhs_fn=lambda k, c: h1r[:, c * TC + 1: (c + 1) * TC + 1, k * 16:(k + 1) * 16])

            # ============ Phase G: L2 recurrence =============
            hb2 = _recurrence(nc, tc, pool, 2, wp["u2"], xg_d[2], None, ds)

            # ============ Phase H: dense head =============
            with nc.named_scope("dense"):
                F2 = LCFG[2]["F"]
                with pool("dps", 1, "PSUM") as dpp:
                    psd = dpp.tile([DENSE, 16], FP32, tag="psd")
                    for k in range(2):
                        nc.tensor.matmul(psd[:], lhsT=wd_sb[:, 64 * k:64 * (k + 1)],
                                         rhs=hb2[:, F2 + 16 * k:F2 + 16 * (k + 1)],
                                         start=(k == 0), stop=(k == 1))
                    hd = constp.tile([DENSE, 16], BF, tag="hd")
                    nc.scalar.activation(hd[:], psd[:], AF.Relu, bias=bd_sb[:, 0:1])
                    psc = dpp.tile([1, 16], FP32, tag="psc")
                    nc.tensor.matmul(psc[:], lhsT=wc_sb[:], rhs=hd[:], start=True, stop=True)
                    outv = constp.tile([1, 16], FP32, tag="outv")
                    nc.scalar.activation(outv[:], psc[:], AF.Sigmoid, bias=bc_sb[0:1, 0:1])
                    nc.sync.dma_start(out_d[:], outv[0:1, :])

    nc.finalize()
    return nc


def _bulk_proj(nc, tc, pool, l, w_dram, bias_sb, xg_dram, rhs_fn):
    """xg_l[:, t*FW + m*16 + b] over a chunked [TC*16]-token loop.
    rhs_fn(k, chunk) -> [128, 512]-sized AP of the (transposed) layer input."""
    import concourse.mybir as mybir
    FP32 = mybir.dt.float32
    BF = mybir.dt.bfloat16
    ALU = mybir.AluOpType
    cfg = LCFG[l]
    nkw, nm, FW = cfg["nkw"], cfg["nm"], cfg["FW"]
    with nc.named_scope(f"xg{l}_bulk"):
        with pool(f"w{l}p", 1) as wpool, pool(f"xps{l}", 2, "PSUM") as xpp, \
                pool(f"stage{l}", 2) as stp:
            w_sb = wpool.tile([128, nkw * nm * 128], BF)
            nc.sync.dma_start(w_sb[:], w_dram[:])
            for c in range(NCHUNK):
                stage = stp.tile([128, TC * FW], FP32, tag="stage")
                stager = stage[:].rearrange("p (t w) -> p t w", w=FW)
                for m in range(nm):
                    ps = xpp.tile([128, 512], FP32, tag="xps")
                    for k in range(nkw):
                        nc.tensor.matmul(
                            ps[:], lhsT=w_sb[:, ((k * nm) + m) * 128:((k * nm) + m + 1) * 128],
                            rhs=rhs_fn(k, c), start=(k == 0), stop=(k == nkw - 1))
                    nc.vector.tensor_scalar(
                        out=stager[:, :, m * 16:(m + 1) * 16],
                        in0=ps[:].rearrange("p (t b) -> p t b", b=16),
                        scalar1=bias_sb[:, m:m + 1], scalar2=None, op0=ALU.add)
                nc.sync.dma_start(xg_dram[:, c * TC * FW:(c + 1) * TC * FW], stage[:])


def _recurrence(nc, tc, pool, l, u_dram, xg_dram, h_seq, ds):
    """Run the T-step LSTM recurrence for layer l. Returns the ping/pong h
    tile (final h in slab p=1). Writes h into h_seq slots 1..T if given."""
    import concourse.mybir as mybir
    FP32 = mybir.dt.float32
    BF = mybir.dt.bfloat16
    AF = mybir.ActivationFunctionType
    ALU = mybir.AluOpType
    cfg = LCFG[l]
    nk, nm, F, FW = cfg["nk"], cfg["nm"], cfg["F"], cfg["FW"]

    with nc.named_scope(f"rec{l}"):
        with pool(f"u{l}p", 1) as upool, pool(f"st{l}", 1) as statep:
            u_sb = upool.tile([128, nk * nm * 128], BF)
            nc.sync.dma_start(u_sb[:], u_dram[:])
            hb = statep.tile([128, 2 * F], BF, tag="hb")
            cbuf = statep.tile([128, F], FP32, tag="cb")
            warm = statep.tile([1, 1], FP32, tag="warm")
            nc.gpsimd.memset(hb[:], 0.0)
            nc.gpsimd.memset(cbuf[:], 0.0)
            if h_seq is not None:
                nc.gpsimd.memset(h_seq[:, 0:F], 0.0)
            # touch the sigmoid table before the loop so the per-iteration
            # ACT_TABLE_LOAD hoists out of the loop body
            nc.scalar.activation(warm[:], cbuf[0:1, 0:1], AF.Sigmoid)

            tc.strict_bb_all_engine_barrier()

            with pool(f"rp{l}a", 2, "PSUM") as ppa, \
                    pool(f"rx{l}", UNROLL + 1) as xgp, pool(f"rg{l}", 2) as gsp, \
                    pool(f"rt{l}", 2) as tmp:
                hint = (mybir.EngineType.PE,) if l == 1 else ()
                with tc.For_i(0, T, UNROLL, staggered_reset=STAGGERED,
                              hint_engines=hint) as i:
                    xgs = []
                    for po in range(UNROLL):
                        xg = xgp.tile([128, FW], FP32, tag="xg")
                        nc.sync.dma_start(xg[:], xg_dram[:, ds((i + po) * FW, FW)])
                        xgs.append(xg)
                    for po in range(UNROLL):
                        p = po % 2
                        t_expr = i + po
                        xg = xgs[po]
                        psa = ppa.tile([128, FW], FP32, tag="psa")
                        for m in range(nm):
                            dst = psa[:, m * 16:(m + 1) * 16]
                            for k in range(nk):
                                nc.tensor.matmul(
                                    dst, lhsT=u_sb[:, ((k * nm) + m) * 128:((k * nm) + m + 1) * 128],
                                    rhs=hb[:, (1 - p) * F + k * 16:(1 - p) * F + (k + 1) * 16],
                                    start=(k == 0), stop=(k == nk - 1))
                        gs = gsp.tile([128, FW], FP32, tag="gs")
                        nc.vector.tensor_add(out=gs[:], in0=psa[:], in1=xg[:])
                        # gates: f=[0:F], i=[F:2F], o=[2F:3F], cc=[3F:4F]
                        sfio = tmp.tile([128, 3 * F], FP32, tag="sfio")
                        nc.scalar.activation(sfio[:], gs[:, 0:3 * F], AF.Sigmoid)
                        c2 = tmp.tile([128, F], FP32, tag="c2")
                        nc.vector.tensor_mul(out=c2[:], in0=cbuf[:], in1=sfio[:, 0:F])
                        t1 = tmp.tile([128, F], FP32, tag="t1")
                        nc.vector.scalar_tensor_tensor(
                            out=t1[:], in0=gs[:, 3 * F:FW], scalar=0.0, in1=sfio[:, F:2 * F],
                            op0=ALU.max, op1=ALU.mult)
                        nc.vector.tensor_add(out=cbuf[:], in0=c2[:], in1=t1[:])
                        nc.vector.scalar_tensor_tensor(
                            out=hb[:, p * F:(p + 1) * F], in0=cbuf[:], scalar=0.0,
                            in1=sfio[:, 2 * F:3 * F], op0=ALU.max, op1=ALU.mult)
                        if h_seq is not None:
                            nc.sync.dma_start(h_seq[:, ds((t_expr + 1) * F, F)],
                                              hb[:, p * F:(p + 1) * F])
            return hb


def _get_program():
    if "nc" not in _CACHE:
        _CACHE["nc"] = build_program()
    return _CACHE["nc"]


def kernel(**inputs):
    global LAST_RESULT
    from concourse.bass_utils import run_bass_kernel_spmd

    nc = _get_program()
    w = prep_weights(inputs)
    tokens = np.asarray(inputs["tokens"], dtype=np.int32)  # [128, 512]

    in_maps = []
    for core in range(NCORES):
        tk = tokens[core * B:(core + 1) * B]          # [16, 512]
        tok_tb = np.ascontiguousarray(tk.T).reshape(-1)  # t-major: idx = t*16+b
        m = {"tokens_tb": tok_tb,
             "emb": np.asarray(inputs["emb"], dtype=np.float32)}
        m.update(w)
        in_maps.append(m)

    trace = os.environ.get("K_TRACE", "0") == "1"
    res = run_bass_kernel_spmd(nc, in_maps, list(range(NCORES)), trace=trace)
    LAST_RESULT = res
    out = np.concatenate([res.results[c]["out"].reshape(B, 1) for c in range(NCORES)], axis=0)
    return out.astype(np.float32)


# revision 15
# speedup vs baseline: 1.2619x; 1.2577x over previous
"""Trainium2 Bass kernel for a 3-layer relu-LSTM classifier.

Architecture (per core, data-parallel over batch across 8 cores, B=16 each):
  x = emb[tokens]                       (indirect-DMA gather, 128 tokens/block)
  xg_l = x_l @ W_l + b_l  (bulk, PE)    -> DRAM, "folded transposed" layout
  recurrence per step (For_i):  g.T = U_l.T @ h.T  (PE, bf16 weights)
      gates/state kept as [128 partitions, nk*16] folded layout so the
      DVE/ACT elementwise ops use all 128 partitions.
  dense head on-device, output sigmoid [16] f32 per core.

Self-contained: hardcodes all shapes; host side only reformats weights
(permutation/fold/bf16 cast) and shards tokens.
"""

import os

import numpy as np
import ml_dtypes

BF16 = ml_dtypes.bfloat16

# Model dims
NCORES = 8
B_TOT, T = 128, 512
B = B_TOT // NCORES  # 16
VOCAB, EMB_D = 5000, 300
EMB_PAD = 384  # padded to 3*128
UNITS = [256, 512, 256]
DENSE = 64

# Per-layer derived dims
# layer l: u, d_in(padded), nk = u//128 (contraction tiles / h fold slabs),
# nm = 4*nk (gate m-tiles), F = nk*16 (fold width), FW = 4*F (g fold width)
LCFG = []
_d = EMB_PAD
for _u in UNITS:
    _nk = _u // 128
    LCFG.append(dict(u=_u, d=_d, nkw=_d // 128, nk=_nk, nm=4 * _nk,
                     F=_nk * 16, FW=4 * _nk * 16))
    _d = _u

TC = 32          # time-steps per bulk-projection chunk (N = TC*16 = 512)
NCHUNK = T // TC
UNROLL = 4       # rec loop half-steps per For_i body (must be even)
STAGGERED = os.environ.get("K_STAGGERED", "1") == "1"

_CACHE = {}
LAST_RESULT = None  # BassKernelResults of the most recent run (for test.py)


def gate_perm(u):
    """Column permutation of [i f cc o]-ordered 4u gate dim into our
    m-tile order: blocks (f, cc, i, o), each block j-minor over u//128."""
    nk = u // 128
    base = [1, 0, 3, 2]  # block order (f, i, o, cc); keras gate idx (i=0, f=1, cc=2, o=3)
    perm = np.empty(4 * u, dtype=np.int64)
    for blk in range(4):
        for j in range(nk):
            m = blk * nk + j
            perm[m * 128:(m + 1) * 128] = base[blk] * u + j * 128 + np.arange(128)
    return perm


def fold_lhs(Wp, nkt, nm):
    """[nkt*128, nm*128] -> [128, nkt*nm*128] with tile (k, m) at cols
    ((k*nm)+m)*128."""
    K, M = Wp.shape
    assert K == nkt * 128 and M == nm * 128, (Wp.shape, nkt, nm)
    return np.ascontiguousarray(
        Wp.reshape(nkt, 128, nm, 128).transpose(1, 0, 2, 3).reshape(128, nkt * nm * 128)
    )


def prep_weights(inputs):
    """Host-side reformatting of the model weights (shared by all cores)."""
    f32 = lambda x: np.asarray(x, dtype=np.float32)
    out = {}
    perms = [gate_perm(u) for u in UNITS]
    W0 = np.zeros((EMB_PAD, 4 * UNITS[0]), np.float32)
    W0[:EMB_D] = f32(inputs["W0"])
    Ws = [W0, f32(inputs["W1"]), f32(inputs["W2"])]
    for l in range(3):
        cfg = LCFG[l]
        p = perms[l]
        out[f"w{l}"] = fold_lhs(Ws[l][:, p], cfg["nkw"], cfg["nm"]).astype(BF16)
        out[f"u{l}"] = fold_lhs(f32(inputs[f"U{l}"])[:, p], cfg["nk"], cfg["nm"]).astype(BF16)
        out[f"b{l}"] = np.ascontiguousarray(
            f32(inputs[f"b{l}"])[p].reshape(cfg["nm"], 128).T)
    Wd = f32(inputs["Wd"])  # [256, 64]
    out["wd"] = np.concatenate([Wd[0:128], Wd[128:256]], axis=1).astype(BF16)  # [128,128]
    out["bd"] = f32(inputs["bd"])           # [64]
    out["wc"] = f32(inputs["Wc"]).astype(BF16)  # [64, 1]
    out["bc"] = f32(inputs["bc"])           # [1]
    return out


def build_program():
    from concourse import bacc
    import concourse.mybir as mybir
    import concourse.tile as tile
    from concourse.bass import ds

    FP32 = mybir.dt.float32
    BF = mybir.dt.bfloat16
    AF = mybir.ActivationFunctionType
    ALU = mybir.AluOpType

    nc = bacc.Bacc(None, target_bir_lowering=False)

    # ---- DRAM parameters ------------------------------------------------
    tok_d = nc.declare_dram_parameter("tokens_tb", [T * B], mybir.dt.int32, isOutput=False)
    emb_d = nc.declare_dram_parameter("emb", [VOCAB, EMB_D], FP32, isOutput=False)
    wp = {}
    for l in range(3):
        cfg = LCFG[l]
        wp[f"w{l}"] = nc.declare_dram_parameter(f"w{l}", [128, cfg["nkw"] * cfg["nm"] * 128], BF, isOutput=False)
        wp[f"u{l}"] = nc.declare_dram_parameter(f"u{l}", [128, cfg["nk"] * cfg["nm"] * 128], BF, isOutput=False)
        wp[f"b{l}"] = nc.declare_dram_parameter(f"b{l}", [128, cfg["nm"]], FP32, isOutput=False)
    wd_d = nc.declare_dram_parameter("wd", [128, 128], BF, isOutput=False)
    bd_d = nc.declare_dram_parameter("bd", [DENSE], FP32, isOutput=False)
    wc_d = nc.declare_dram_parameter("wc", [DENSE, 1], BF, isOutput=False)
    bc_d = nc.declare_dram_parameter("bc", [1], FP32, isOutput=False)
    out_d = nc.declare_dram_parameter("out", [B], FP32, isOutput=True)

    # ---- internal DRAM scratch (padded for recurrence xg prefetch) ------
    xg_d = [nc.dram_tensor(f"xg{l}", [128, (T + 4 * UNROLL) * LCFG[l]["FW"]], BF)
            for l in range(3)]

    from concourse.masks import make_identity

    with tile.TileContext(nc) as tc:
        stk = []

        def pool(name, bufs, space="SBUF"):
            return tc.tile_pool(name=name, bufs=bufs, space=space)

        with pool("const", 1) as constp:
            ident = constp.tile([128, 128], FP32)
            make_identity(nc, ident[:])
            identb = constp.tile([128, 128], BF)
            make_identity(nc, identb[:])
            tok_sb = constp.tile([128, (T * B) // 128], mybir.dt.int32)
            nc.sync.dma_start(tok_sb[:], tok_d[:].rearrange("(i p) -> p i", p=128))
            bias_sb = []
            for l in range(3):
                bt = constp.tile([128, LCFG[l]["nm"]], FP32, tag=f"bias{l}")
                nc.sync.dma_start(bt[:], wp[f"b{l}"][:])
                bias_sb.append(bt)
            wd_sb = constp.tile([128, 128], BF)
            nc.sync.dma_start(wd_sb[:], wd_d[:])
            bd_sb = constp.tile([DENSE, 1], FP32)
            nc.sync.dma_start(bd_sb[:], bd_d[:])
            wc_sb = constp.tile([DENSE, 1], BF)
            nc.sync.dma_start(wc_sb[:], wc_d[:])
            bc_sb = constp.tile([1, 1], FP32)
            nc.sync.dma_start(bc_sb[:], bc_d[:])

            # ============ Phase A: gather + transpose -> xT =============
            NTOK = T * B           # 8192
            NBLK = NTOK // 128     # 64
            with pool("xT", 1) as xtp:
                xT = xtp.tile([128, 3 * NTOK], BF)
                # zero slab k=2 (rows 44: stay zero; 0:44 overwritten below)
                nc.gpsimd.memset(xT[:, 2 * NTOK:3 * NTOK], 0.0)
                with nc.named_scope("gather_transpose"):
                    with pool("gath", 3) as gp, pool("tps", 2, "PSUM") as tpp:
                        for blk in range(NBLK):
                            xb = gp.tile([128, EMB_PAD], FP32, tag="xb")
                            import concourse.bass as bass_mod
                            nc.gpsimd.indirect_dma_start(
                                out=xb[:, 0:EMB_D], out_offset=None,
                                in_=emb_d[:, :],
                                in_offset=bass_mod.IndirectOffsetOnAxis(
                                    ap=tok_sb[:, blk:blk + 1], axis=0),
                            )
                            for k in range(3):
                                tps = tpp.tile([128, 128], FP32, tag="tps")
                                nc.tensor.transpose(tps[:], xb[:, 128 * k:128 * (k + 1)], ident[:])
                                rows = 128 if k < 2 else 44
                                nc.vector.tensor_copy(
                                    out=xT[0:rows, k * NTOK + 128 * blk: k * NTOK + 128 * (blk + 1)],
                                    in_=tps[0:rows, :])

                # ============ Phase B: xg0 bulk =============
                _bulk_proj(nc, tc, pool, 0, wp["w0"], bias_sb[0], xg_d[0],
                           rhs_fn=lambda k, c: xT[:, k * NTOK + c * 512: k * NTOK + (c + 1) * 512])

            # ============ Phase C: L0 recurrence =============
            with pool("seq0", 1) as sq0:
                h0_seq = sq0.tile([128, (T + 1) * LCFG[0]["F"]], BF)
                _recurrence(nc, tc, pool, 0, wp["u0"], xg_d[0], h0_seq, ds, identb)

                # ============ Phase D: xg1 bulk =============
                F0 = LCFG[0]["F"]
                h0r = h0_seq[:].rearrange("p (s w) -> p s w", w=F0)
                _bulk_proj(nc, tc, pool, 1, wp["w1"], bias_sb[1], xg_d[1],
                           rhs_fn=lambda k, c: h0r[:, c * TC + 1: (c + 1) * TC + 1, k * 16:(k + 1) * 16])

            # ============ Phase E: L1 recurrence =============
            with pool("seq1", 1) as sq1:
                h1_seq = sq1.tile([128, (T + 1) * LCFG[1]["F"]], BF)
                _recurrence(nc, tc, pool, 1, wp["u1"], xg_d[1], h1_seq, ds, identb)

                # ============ Phase F: xg2 bulk =============
                F1 = LCFG[1]["F"]
                h1r = h1_seq[:].rearrange("p (s w) -> p s w", w=F1)
                _bulk_proj(nc, tc, pool, 2, wp["w2"], bias_sb[2], xg_d[2],
                           rhs_fn=lambda k, c: h1r[:, c * TC + 1: (c + 1) * TC + 1, k * 16:(k + 1) * 16])

            # ============ Phase G: L2 recurrence =============
            hb2 = _recurrence(nc, tc, pool, 2, wp["u2"], xg_d[2], None, ds, identb)

            # ============ Phase H: dense head =============
            with nc.named_scope("dense"):
                F2 = LCFG[2]["F"]
                with pool("dps", 1, "PSUM") as dpp:
                    psd = dpp.tile([DENSE, 16], FP32, tag="psd")
                    for k in range(2):
                        nc.tensor.matmul(psd[:], lhsT=wd_sb[:, 64 * k:64 * (k + 1)],
                                         rhs=hb2[:, F2 + 16 * k:F2 + 16 * (k + 1)],
                                         start=(k == 0), stop=(k == 1))
                    hd = constp.tile([DENSE, 16], BF, tag="hd")
                    nc.scalar.activation(hd[:], psd[:], AF.Relu, bias=bd_sb[:, 0:1])
                    psc = dpp.tile([1, 16], FP32, tag="psc")
                    nc.tensor.matmul(psc[:], lhsT=wc_sb[:], rhs=hd[:], start=True, stop=True)
                    outv = constp.tile([1, 16], FP32, tag="outv")
                    nc.scalar.activation(outv[:], psc[:], AF.Sigmoid, bias=bc_sb[0:1, 0:1])
                    nc.sync.dma_start(out_d[:], outv[0:1, :])

    nc.finalize()
    return nc


def _bulk_proj(nc, tc, pool, l, w_dram, bias_sb, xg_dram, rhs_fn):
    """xg_l[:, t*FW + m*16 + b] over a chunked [TC*16]-token loop.
    rhs_fn(k, chunk) -> [128, 512]-sized AP of the (transposed) layer input."""
    import concourse.mybir as mybir
    FP32 = mybir.dt.float32
    BF = mybir.dt.bfloat16
    ALU = mybir.AluOpType
    cfg = LCFG[l]
    nkw, nm, FW = cfg["nkw"], cfg["nm"], cfg["FW"]
    with nc.named_scope(f"xg{l}_bulk"):
        with pool(f"w{l}p", 1) as wpool, pool(f"xps{l}", 2, "PSUM") as xpp, \
                pool(f"stage{l}", 2) as stp:
            w_sb = wpool.tile([128, nkw * nm * 128], BF)
            nc.sync.dma_start(w_sb[:], w_dram[:])
            for c in range(NCHUNK):
                stage = stp.tile([128, TC * FW], BF, tag="stage")
                stager = stage[:].rearrange("p (t w) -> p t w", w=FW)
                for m in range(nm):
                    ps = xpp.tile([128, 512], FP32, tag="xps")
                    for k in range(nkw):
                        nc.tensor.matmul(
                            ps[:], lhsT=w_sb[:, ((k * nm) + m) * 128:((k * nm) + m + 1) * 128],
                            rhs=rhs_fn(k, c), start=(k == 0), stop=(k == nkw - 1))
                    nc.vector.tensor_scalar(
                        out=stager[:, :, m * 16:(m + 1) * 16],
                        in0=ps[:].rearrange("p (t b) -> p t b", b=16),
                        scalar1=bias_sb[:, m:m + 1], scalar2=None, op0=ALU.add)
                nc.sync.dma_start(xg_dram[:, c * TC * FW:(c + 1) * TC * FW], stage[:])


def _recurrence(nc, tc, pool, l, u_dram, xg_dram, h_seq, ds, identb):
    """Run the T-step LSTM recurrence for layer l. Returns the ping/pong h
    tile (final h in slab p=1). Writes h into h_seq slots 1..T if given.

    Body covers 2*UNROLL steps with two xg prefetch buffers (A/B) so the
    xg slab DMA is always a body ahead. The per-step xg add is injected
    into the PSUM accumulation via an identity-stationary matmul; sigmoid
    reads PSUM directly."""
    import concourse.mybir as mybir
    FP32 = mybir.dt.float32
    BF = mybir.dt.bfloat16
    AF = mybir.ActivationFunctionType
    ALU = mybir.AluOpType
    cfg = LCFG[l]
    nk, nm, F, FW = cfg["nk"], cfg["nm"], cfg["F"], cfg["FW"]
    U2 = 2 * UNROLL  # steps per body

    with nc.named_scope(f"rec{l}"):
        with pool(f"u{l}p", 1) as upool, pool(f"st{l}", 1) as statep:
            u_sb = upool.tile([128, nk * nm * 128], BF)
            nc.sync.dma_start(u_sb[:], u_dram[:])
            hb = statep.tile([128, 2 * F], BF, tag="hb")
            cbuf = statep.tile([128, F], FP32, tag="cb")
            warm = statep.tile([1, 1], FP32, tag="warm")
            xga = statep.tile([128, UNROLL * FW], BF, tag="xga")
            xgb = statep.tile([128, UNROLL * FW], BF, tag="xgb")
            nc.gpsimd.memset(hb[:], 0.0)
            nc.gpsimd.memset(cbuf[:], 0.0)
            if h_seq is not None:
                nc.gpsimd.memset(h_seq[:, 0:F], 0.0)
            # touch the sigmoid table before the loop so the per-iteration
            # ACT_TABLE_LOAD hoists out of the loop body
            nc.scalar.activation(warm[:], cbuf[0:1, 0:1], AF.Sigmoid)
            # preload first half-body's xg slabs
            nc.sync.dma_start(xga[:], xg_dram[:, 0:UNROLL * FW])

            tc.strict_bb_all_engine_barrier()

            def step(po, xgbuf):
                p = po % 2
                sl = po % UNROLL
                ps_fio = ppa.tile([128, 3 * F], FP32, tag="psfio")
                ps_cc = ppc.tile([128, F], FP32, tag="pscc")
                xg_sl = xgbuf[:, sl * FW:sl * FW + FW]
                # inject xg into PSUM, then accumulate U k-tiles
                nc.tensor.matmul(ps_fio[:], lhsT=identb[:], rhs=xg_sl[:, 0:3 * F],
                                 start=True, stop=False, skip_group_check=True)
                for m in range(3 * nk):
                    dst = ps_fio[:, m * 16:(m + 1) * 16]
                    for k in range(nk):
                        nc.tensor.matmul(
                            dst, lhsT=u_sb[:, ((k * nm) + m) * 128:((k * nm) + m + 1) * 128],
                            rhs=hb[:, (1 - p) * F + k * 16:(1 - p) * F + (k + 1) * 16],
                            start=False, stop=(k == nk - 1), skip_group_check=True)
                nc.tensor.matmul(ps_cc[:], lhsT=identb[:], rhs=xg_sl[:, 3 * F:FW],
                                 start=True, stop=False, skip_group_check=True)
                for m in range(3 * nk, nm):
                    dst = ps_cc[:, (m - 3 * nk) * 16:(m - 3 * nk + 1) * 16]
                    for k in range(nk):
                        nc.tensor.matmul(
                            dst, lhsT=u_sb[:, ((k * nm) + m) * 128:((k * nm) + m + 1) * 128],
                            rhs=hb[:, (1 - p) * F + k * 16:(1 - p) * F + (k + 1) * 16],
                            start=False, stop=(k == nk - 1), skip_group_check=True)
                # gates: f=[0:F], i=[F:2F], o=[2F:3F] in ps_fio; cc in ps_cc
                sfio = tmp.tile([128, 3 * F], FP32, tag="sfio")
                nc.scalar.activation(sfio[:], ps_fio[:], AF.Sigmoid)
                c2 = tmp.tile([128, F], FP32, tag="c2")
                nc.gpsimd.tensor_mul(out=c2[:], in0=cbuf[:], in1=sfio[:, 0:F])
                t1 = tmp.tile([128, F], FP32, tag="t1")
                nc.vector.scalar_tensor_tensor(
                    out=t1[:], in0=ps_cc[:], scalar=0.0, in1=sfio[:, F:2 * F],
                    op0=ALU.max, op1=ALU.mult)
                nc.vector.tensor_add(out=cbuf[:], in0=c2[:], in1=t1[:])
                nc.vector.scalar_tensor_tensor(
                    out=hb[:, p * F:(p + 1) * F], in0=cbuf[:], scalar=0.0,
                    in1=sfio[:, 2 * F:3 * F], op0=ALU.max, op1=ALU.mult)

            def hflush(i, po):
                # after odd steps: slots (i+po, i+po+1) = hb[0:2F]
                if h_seq is not None:
                    nc.gpsimd.dma_start(h_seq[:, ds((i + po) * F, 2 * F)], hb[:])

            with pool(f"rp{l}a", 2, "PSUM") as ppa, pool(f"rp{l}c", 2, "PSUM") as ppc, \
                    pool(f"rt{l}", 2) as tmp:
                hint = (mybir.EngineType.PE, mybir.EngineType.SP)
                with tc.For_i(0, T, U2, staggered_reset=STAGGERED,
                              hint_engines=hint) as i:
                    # A holds [i, i+U); prefetch [i+U, i+2U) into B, compute
                    # from A; reload A <- [i+2U, i+3U) for the next body.
                    nc.sync.dma_start(xgb[:], xg_dram[:, ds((i + UNROLL) * FW, UNROLL * FW)])
                    for po in range(UNROLL):
                        step(po, xga)
                        if po % 2 == 1:
                            hflush(i, po)
                    nc.sync.dma_start(xga[:], xg_dram[:, ds((i + 2 * UNROLL) * FW, UNROLL * FW)])
                    for po in range(UNROLL, U2):
                        step(po, xgb)
                        if po % 2 == 1:
                            hflush(i, po)
            return hb


def _get_program():
    if "nc" not in _CACHE:
        _CACHE["nc"] = build_program()
    return _CACHE["nc"]


def kernel(**inputs):
    global LAST_RESULT
    from concourse.bass_utils import run_bass_kernel_spmd

    nc = _get_program()
    w = prep_weights(inputs)
    tokens = np.asarray(inputs["tokens"], dtype=np.int32)  # [128, 512]

    in_maps = []
    for core in range(NCORES):
        tk = tokens[core * B:(core + 1) * B]          # [16, 512]
        tok_tb = np.ascontiguousarray(tk.T).reshape(-1)  # t-major: idx = t*16+b
        m = {"tokens_tb": tok_tb,
             "emb": np.asarray(inputs["emb"], dtype=np.float32)}
        m.update(w)
        in_maps.append(m)

    trace = os.environ.get("K_TRACE", "0") == "1"
    res = run_bass_kernel_spmd(nc, in_maps, list(range(NCORES)), trace=trace)
    LAST_RESULT = res
    out = np.concatenate([res.results[c]["out"].reshape(B, 1) for c in range(NCORES)], axis=0)
    return out.astype(np.float32)
